# revision 16
# baseline (speedup 1.0000x reference)
"""Trainium2 Bass kernel for LorentzMultiheadAttention (B=2, N=2048, H=8, D=64, E=512).

Sharding: 8 cores = 2 batches x 4 head-pairs. Core c handles batch b=c//4 and
heads {2*(c%4), 2*(c%4)+1}. Queries are processed in 2 halves of 1024; each
half's per-head centroid + head-sum is ReduceScattered over the 4-core batch
group while the other half computes, hiding most of the collective.

Key optimizations vs the bf16 baseline:
- All heavy matmuls are fp8e4 DoubleRow (2 rows/cycle): projections pair the
  four E=512/128 contraction subtiles; PV pairs adjacent key tiles; scores
  (contract=64 only) pair the real K/Q subtile with a zero subtile, which
  still streams queries at 2 cols/cycle.
- Everything is scaled by 32 (weights) so fp8e4 stays in normal range. The
  scale cancels: the Lorentz centroid is scale-invariant, the lift becomes
  t = sqrt(32^2 + ||x'||^2), and the score scale folds into the exp activation
  scale (S'' = 1024 * S').
- The softmax exp writes fp8 directly (ACT output cast), feeding PV with no
  extra conversion; softmax normalization and the mean-over-heads divide are
  skipped entirely (centroid scale-invariance).
- Attention output stays in the transposed [d, q] layout through both
  centroids, so there are no PE transposes in the epilogue; rsqrt is computed
  as exp(-0.5*ln(x)) to keep the ACT table in the natural_log_exp set (one
  table switch after the phase-A sqrt epoch).
- Lorentz sign folded into negated K weights (host-side).
"""

import os
import sys

for _p in ("/opt/trn_rl_repo", "/root/.axon_site/_ro/trn_rl_repo"):
    if os.path.isdir(_p) and _p not in sys.path:
        sys.path.insert(0, _p)

import numpy as np

import concourse.bacc as bacc
import concourse.bass as bass
import concourse.mybir as mybir
import concourse.tile as tile

B = 2
N = 2048
H = 8
D = 64
E = 512
DM1 = D - 1  # 63
P = 128
N_CORES = 8
HPC = 2   # heads per core
NH = 1024  # queries per half
QB = 256  # query block each core receives from one half's ReduceScatter

W_SCALE = 32.0  # weight scaling so fp8e4 stays in normal range
SC2 = W_SCALE * W_SCALE  # 1024

F32 = mybir.dt.float32
BF16 = mybir.dt.bfloat16
FP8 = mybir.dt.float8e4
EXP = mybir.ActivationFunctionType.Exp
LN = mybir.ActivationFunctionType.Ln
SQRT = mybir.ActivationFunctionType.Sqrt
ADD = mybir.AluOpType.add
MULT = mybir.AluOpType.mult
DR = mybir.MatmulPerfMode.DoubleRow

REPLICA_GROUPS = [[0, 1, 2, 3], [4, 5, 6, 7]]


def _emit(tc, nc, io, scale_val, bias_val):
    from contextlib import ExitStack

    ctx = ExitStack()
    with ctx:
        consts = ctx.enter_context(tc.tile_pool(name="consts", bufs=1))
        sb = ctx.enter_context(tc.tile_pool(name="sb", bufs=1))

        # ---- constants / weights to SBUF ----
        w_sb = {}
        for nm in ("wq", "wk", "wv"):
            w = consts.tile([P, 4, P], FP8, name=f"{nm}_sb")
            nc.sync.dma_start(w[:], io[nm].ap().rearrange("(c p) m -> p c m", p=P))
            w_sb[nm] = w
        b_sb = {}
        for nm in ("bq", "bk"):
            bt = consts.tile([P, 1], F32, name=f"{nm}_sb")
            nc.sync.dma_start(bt[:], io[nm].ap().rearrange("(p one) -> p one", one=1))
            b_sb[nm] = bt
        lift_mask = consts.tile([P, P], BF16)
        nc.sync.dma_start(lift_mask[:], io["lift_mask"].ap())
        sc2bias = consts.tile([P, 1], F32)
        nc.gpsimd.memset(sc2bias[:], SC2)
        ebias = consts.tile([P, 1], F32)

        # ---- inputs (fp8, chunked DMA so projections can start early) ----
        xs = sb.tile([P, 4, N], FP8)
        xq = sb.tile([P, 4, N], FP8)
        for ec in range(4):
            nc.sync.dma_start(
                xs[:, ec : ec + 1, :],
                io["xs"].ap().rearrange("(c p) n -> p c n", p=P)[:, ec : ec + 1, :],
            )
        for ec in range(4):
            nc.sync.dma_start(
                xq[:, ec : ec + 1, :],
                io["xq"].ap().rearrange("(c p) n -> p c n", p=P)[:, ec : ec + 1, :],
            )

        # q/k layout: [128 (h*64+d), 2 (real, zero-pad), 2048] fp8
        q_sb = sb.tile([P, 2, N], FP8)
        k_sb = sb.tile([P, 2, N], FP8)
        nc.vector.memset(q_sb[:, 1, :], 0.0)
        nc.vector.memset(k_sb[:, 1, :], 0.0)
        # v layout: [128 keys, 8 pairs, 2 (mc in pair), 128 (h*64+d)] fp8
        v_sb = sb.tile([P, 8, 2, P], FP8)

        ctxA = ExitStack()
        psP = ctxA.enter_context(tc.tile_pool(name="psP", bufs=1, space="PSUM"))
        psNRM = ctxA.enter_context(tc.tile_pool(name="psNRM", bufs=1, space="PSUM"))

        def project_T(x_sb, wname, bias, dst):
            """Transposed projection: dst[:, 0, :] = W^T x + b (both heads)."""
            ps = psP.tile([P, 4, 512], F32, tag="projps")
            for pr in range(2):
                for qc in range(4):
                    nc.tensor.matmul(
                        ps[:, qc, :],
                        lhsT=w_sb[wname][:, 2 * pr : 2 * pr + 2, :],
                        rhs=x_sb[:, 2 * pr : 2 * pr + 2, qc * 512 : (qc + 1) * 512],
                        start=(pr == 0),
                        stop=(pr == 1),
                        perf_mode=DR,
                    )
            nc.vector.tensor_tensor(
                dst[:, 0, :],
                ps[:].rearrange("p c n -> p (c n)"),
                bias[:].to_broadcast((P, N)),
                ADD,
            )

        def lift_T(dst):
            """Write t = sqrt(1024 + ||x_s||^2) into rows 0/64 of dst[:,0,:].

            The norm matmul lands both heads' norms on partitions {0,1}
            (engines can't use strided partition APs), the sqrt writes an fp8
            staging row-pair, and a DMA scatters it to partitions {0,64}."""
            sq = sb.tile([P, N], BF16, tag="liftsq")
            nc.vector.tensor_tensor(sq[:], dst[:, 0, :], dst[:, 0, :], MULT)
            nrm = psNRM.tile([2, 4, 512], F32, tag="nrmps")
            for qc in range(4):
                nc.tensor.matmul(
                    nrm[:, qc, :],
                    lhsT=lift_mask[:, 0:2],
                    rhs=sq[:, qc * 512 : (qc + 1) * 512],
                    start=True,
                    stop=True,
                )
            ttmp = sb.tile([2, N], FP8, tag="ttmp")
            nc.scalar.activation(
                ttmp[:],
                nrm[0:2, :, :].rearrange("h c n -> h (c n)"),
                SQRT,
                bias=sc2bias[0:2, :],
                scale=1.0,
            )
            nc.sync.dma_start(
                dst[:].rearrange("(h d) s n -> h d s n", h=2)[:, 0, 0, :],
                ttmp[:],
            )

        project_T(xs, "wk", b_sb["bk"], k_sb)
        lift_T(k_sb)
        project_T(xq, "wq", b_sb["bq"], q_sb)
        lift_T(q_sb)

        # ---- V in natural layout [keys, h*64+d] via DoubleRow, lift rows ----
        vps = psP.tile([P, 4, 4, P], F32, tag="projps")  # [keys, (4mc/bank), hd]
        for pr_e in range(2):
            for mc in range(16):
                nc.tensor.matmul(
                    vps[:, mc // 4, mc % 4, :],
                    lhsT=xs[:, 2 * pr_e : 2 * pr_e + 2, mc * P : (mc + 1) * P],
                    rhs=w_sb["wv"][:, 2 * pr_e : 2 * pr_e + 2, :],
                    start=(pr_e == 0),
                    stop=(pr_e == 1),
                    perf_mode=DR,
                )
        nc.vector.tensor_copy(
            out=v_sb[:].rearrange("p a b m -> p (a b m)"),
            in_=vps[:].rearrange("p a b m -> p (a b m)"),
        )
        vsq = sb.tile([P, 8, 2, P], BF16, tag="vsq")
        nc.vector.tensor_tensor(vsq[:], v_sb[:], v_sb[:], MULT)
        vn = sb.tile([P, 8, 2, 2, 1], F32, tag="vn")
        nc.vector.tensor_reduce(
            vn[:, :, :, :, 0],
            vsq[:].rearrange("p a b (h d) -> p a b h d", h=2),
            axis=mybir.AxisListType.X,
            op=ADD,
        )
        nc.scalar.activation(
            v_sb[:].rearrange("p a b (h d) -> p a b h d", h=2)[:, :, :, :, 0:1],
            vn[:],
            SQRT,
            bias=sc2bias[:],
            scale=1.0,
        )
        # ebias = 0*v_sb[:,0,0,0] + act_bias: pins every exp behind the V lift
        # so the ACT queue runs all three Sqrt calls before switching to the
        # Exp table (avoids a second sqrt/exp table-load pair).
        nc.vector.tensor_scalar(
            ebias[:], v_sb[:, 0, 0, 0:1], 0.0, 2.0 / scale_val + bias_val,
            MULT, ADD,
        )
        ctxA.close()

        # ---- Phase B/C: attention + chunked centroid/ReduceScatter ----
        pP = ctx.enter_context(tc.tile_pool(name="pP", bufs=2))
        dram = ctx.enter_context(tc.tile_pool(name="dram", bufs=1, space="DRAM"))
        psS = ctx.enter_context(tc.tile_pool(name="psS", bufs=2, space="PSUM"))
        psPV = ctx.enter_context(tc.tile_pool(name="psPV", bufs=1, space="PSUM"))

        act_scale = -2.0 / (scale_val * SC2)
        act_bias = 2.0 / scale_val + bias_val

        cc_outs = []
        for half in range(2):
            q0 = half * NH
            pv = psPV.tile([64, 2, 2, 512], F32, tag="pv")
            pend_pv = None

            def flush_pv(args):
                pr, h, p_t = args
                for qcl in range(2):
                    nc.tensor.matmul(
                        pv[:, h, qcl, :],
                        lhsT=v_sb[:, pr, :, h * 64 : (h + 1) * 64],
                        rhs=p_t[:, :, qcl * 512 : (qcl + 1) * 512],
                        start=(pr == 0),
                        stop=(pr == 7),
                        perf_mode=DR,
                        skip_group_check=True,
                    )

            for pr in range(8):
                for h in range(2):
                    p_t = pP.tile([P, 2, NH], FP8, tag="p")
                    for mci in range(2):
                        mc = 2 * pr + mci
                        s_ps = psS.tile([P, NH], F32, tag="s")
                        for qcl in range(2):
                            nc.tensor.matmul(
                                s_ps[:, qcl * 512 : (qcl + 1) * 512],
                                lhsT=k_sb[
                                    h * 64 : (h + 1) * 64, :, mc * P : (mc + 1) * P
                                ],
                                rhs=q_sb[
                                    h * 64 : (h + 1) * 64,
                                    :,
                                    q0 + qcl * 512 : q0 + (qcl + 1) * 512,
                                ],
                                start=True,
                                stop=True,
                                perf_mode=DR,
                            )
                        nc.scalar.activation(
                            p_t[:, mci, :], s_ps[:], EXP, scale=act_scale,
                            bias=ebias[:],
                        )
                    if pend_pv is not None:
                        flush_pv(pend_pv)
                    pend_pv = (pr, h, p_t)
            flush_pv(pend_pv)

            # ---- per-head centroid + head-sum (natural [q, d] layout) ----
            # pv is [64 d, (h, qcl), 512 q]; DMA-transpose to [128 q, qt, hd].
            o_bf = sb.tile([64, 2, 2, 512], BF16, tag="o_bf")
            nc.vector.tensor_copy(
                out=o_bf[:].rearrange("p a b n -> p (a b n)"),
                in_=pv[:].rearrange("p a b n -> p (a b n)"),
            )
            o_nat = sb.tile([P, 8, P], BF16, tag="o_nat")
            for h in range(2):
                for qcl in range(2):
                    for c in range(4):
                        nc.sync.dma_start_transpose(
                            o_nat[:, qcl * 4 + c, h * 64 : (h + 1) * 64],
                            o_bf[:, h, qcl, c * P : (c + 1) * P],
                        )
            sq = sb.tile([P, 8, P], BF16, tag="sq_nat")
            nc.vector.tensor_tensor(sq[:], o_nat[:], o_nat[:], MULT)
            p2 = sb.tile([P, 8, 2, 1], F32, tag="p2")
            nc.vector.tensor_reduce(
                p2[:, :, :, 0],
                sq[:].rearrange("p a (h d) -> p a h d", h=2),
                axis=mybir.AxisListType.X,
                op=ADD,
            )
            tbar = sb.tile([P, 8, 2, 1], F32, tag="tbar")
            nc.vector.tensor_copy(
                out=tbar[:].rearrange("p a h one -> p (a h one)"),
                in_=o_nat[:].rearrange("p a (h d) -> p a h d", h=2)[:, :, :, 0:1]
                .rearrange("p a h one -> p (a h one)"),
            )
            # delta = colsum - t^2, using the *same* bf16-rounded t^2 from sq
            # so the big terms cancel exactly.
            delta = sb.tile([P, 8, 2, 1], F32, tag="delta")
            fl = lambda ap: ap.rearrange("p a h one -> p (a h) one")
            nc.vector.affine_then_add(
                out=fl(delta[:]),
                in0=sq[:].rearrange("p a (h d) -> p (a h) d", h=2)[:, :, 0:1],
                in1=fl(p2[:]),
                scale=-1.0,
                bias=0.0,
            )
            rt = sb.tile([P, 8, 2, 1], F32, tag="rt")
            nc.vector.reciprocal_approx_fast(fl(rt[:]), fl(tbar[:]))
            z = sb.tile([P, 8, 2, 1], F32, tag="z")
            nc.vector.tensor_tensor(z[:], delta[:], rt[:], MULT)
            den = sb.tile([P, 8, 2, 1], F32, tag="den")
            nc.vector.affine_then_add(
                out=fl(den[:]), in0=fl(z[:]), in1=fl(tbar[:]), scale=-0.5, bias=0.0
            )
            rec = sb.tile([P, 8, 2, 1], F32, tag="rec")
            nc.vector.reciprocal_approx_fast(fl(rec[:]), fl(den[:]))
            # out_head = o_nat * rec (free-dim broadcast), then sum heads
            o4 = o_nat[:].rearrange("p a (h d) -> p a h d", h=2)
            m0 = sb.tile([P, 8, 64], F32, tag="m0")
            nc.vector.tensor_tensor(
                m0[:], o4[:, :, 0, :], rec[:, :, 0, :].to_broadcast((P, 8, 64)), MULT
            )
            m1 = sb.tile([P, 8, 64], F32, tag="m1")
            nc.vector.tensor_tensor(
                m1[:], o4[:, :, 1, :], rec[:, :, 1, :].to_broadcast((P, 8, 64)), MULT
            )
            hsum = sb.tile([P, 8, 64], F32, tag="hsum")
            nc.vector.tensor_tensor(hsum[:], m0[:], m1[:], ADD)

            # ---- ReduceScatter this half over the 4-core batch group ----
            cc_in = dram.tile([4, QB, 64], F32, name=f"cc_in{half}")
            cc_out = dram.tile([QB, 64], F32, name=f"cc_out{half}")
            nc.sync.dma_start(
                cc_in[:].rearrange("g (a p) d -> p g a d", p=P),
                hsum[:].rearrange("p (g a) d -> p g a d", g=4),
            )
            nc.gpsimd.collective_compute(
                "ReduceScatter",
                ADD,
                replica_groups=REPLICA_GROUPS,
                ins=[cc_in[:].opt()],
                outs=[cc_out[:].opt()],
            )
            cc_outs.append(cc_out)

        # ---- final centroid on the two local 256-query slices (natural) ----
        for half in range(2):
            fin = sb.tile([P, 2, 64], F32, tag="fin")
            nc.sync.dma_start(
                fin[:], cc_outs[half][:].rearrange("(a p) d -> p a d", p=P)
            )
            fsq = sb.tile([P, 2, 64], BF16, tag="fsq")
            nc.vector.tensor_tensor(fsq[:], fin[:], fin[:], MULT)
            fp2 = sb.tile([P, 2, 1], F32, tag="fp2")
            nc.vector.tensor_reduce(
                fp2[:, :, 0], fsq[:], axis=mybir.AxisListType.X, op=ADD
            )
            fdel = sb.tile([P, 2, 1], F32, tag="fdel")
            nc.vector.affine_then_add(
                out=fdel[:], in0=fsq[:, :, 0:1], in1=fp2[:], scale=-1.0, bias=0.0
            )
            frt = sb.tile([P, 2, 1], F32, tag="frt")
            nc.vector.reciprocal_approx_fast(frt[:], fin[:, :, 0:1])
            fz = sb.tile([P, 2, 1], F32, tag="fz")
            nc.vector.tensor_tensor(fz[:], fdel[:], frt[:], MULT)
            fden = sb.tile([P, 2, 1], F32, tag="fden")
            nc.vector.affine_then_add(
                out=fden[:], in0=fz[:], in1=fin[:, :, 0:1], scale=-0.5, bias=0.0
            )
            frec = sb.tile([P, 2, 1], F32, tag="frec")
            nc.vector.reciprocal_approx_fast(frec[:], fden[:])
            out_sb = sb.tile([P, 2, 64], F32, tag="out_sb")
            nc.vector.tensor_tensor(
                out_sb[:], fin[:], frec[:].to_broadcast((P, 2, 64)), MULT
            )
            nc.sync.dma_start(
                io["out"].ap()[half, :, :].rearrange("(a p) d -> p a d", p=P),
                out_sb[:],
            )


def _build(scale_val, bias_val):
    nc = bacc.Bacc(num_devices=N_CORES)
    io = {}
    io["xq"] = nc.declare_dram_parameter("xq", [E, N], FP8, isOutput=False)
    io["xs"] = nc.declare_dram_parameter("xs", [E, N], FP8, isOutput=False)
    for nm in ("wq", "wk", "wv"):
        io[nm] = nc.declare_dram_parameter(nm, [E, P], FP8, isOutput=False)
    for nm in ("bq", "bk"):
        io[nm] = nc.declare_dram_parameter(nm, [P], F32, isOutput=False)
    io["lift_mask"] = nc.declare_dram_parameter("lift_mask", [P, P], BF16, isOutput=False)
    io["out"] = nc.declare_dram_parameter("out", [2, QB, 64], F32, isOutput=True)

    with tile.TileContext(nc) as tc:
        _emit(tc, nc, io, scale_val, bias_val)
    nc.compile()
    return nc


_BUILD_CACHE = {}


def _get_nc(scale_val, bias_val):
    key = (float(scale_val), float(bias_val))
    if key not in _BUILD_CACHE:
        _BUILD_CACHE[key] = _build(*key)
    return _BUILD_CACHE[key]


def _pad_wT(w_heads):
    """w_heads: [126, 512] spatial weights for 2 heads -> [512, 128] transposed
    with zero columns at 0 and 64 (time slots), scaled by W_SCALE."""
    out = np.zeros((E, P), dtype=np.float32)
    out[:, 1:64] = W_SCALE * w_heads[0:DM1, :].T
    out[:, 65:128] = W_SCALE * w_heads[DM1 : 2 * DM1, :].T
    return np.ascontiguousarray(out)


def _pad_b(b_heads):
    out = np.zeros((P,), dtype=np.float32)
    out[1:64] = W_SCALE * b_heads[0:DM1]
    out[65:128] = W_SCALE * b_heads[DM1 : 2 * DM1]
    return out


def make_in_maps(
    query_input, source_input, Wq_w, Wq_b, Wk_w, Wk_b, Wv_w, Wv_b, scale, bias
):
    import ml_dtypes

    F8 = ml_dtypes.float8_e4m3fn
    BF = ml_dtypes.bfloat16

    lift_mask = np.zeros((P, P), dtype=np.float32)
    lift_mask[1:64, 0] = 1.0
    lift_mask[65:128, 1] = 1.0

    in_maps = []
    for c in range(N_CORES):
        b = c // 4
        h0 = 2 * (c % 4)
        sl = slice(h0 * DM1, (h0 + 2) * DM1)
        m = {
            "xq": np.ascontiguousarray(query_input[b].T).astype(F8),
            "xs": np.ascontiguousarray(source_input[b].T).astype(F8),
            "wq": _pad_wT(Wq_w[sl]).astype(F8),
            "wk": _pad_wT(-Wk_w[sl]).astype(F8),  # Lorentz sign folded into K
            "wv": _pad_wT(Wv_w[sl]).astype(F8),
            "bq": _pad_b(Wq_b[sl]),
            "bk": _pad_b(-Wk_b[sl]),
            "lift_mask": lift_mask.astype(BF),
        }
        in_maps.append(m)
    return in_maps


def kernel(
    query_input,
    source_input,
    Wq_w,
    Wq_b,
    Wk_w,
    Wk_b,
    Wv_w,
    Wv_b,
    scale,
    bias,
    _trace=False,
):
    assert not np.any(np.asarray(Wv_b)), "nonzero V bias not supported"
    scale_val = float(np.asarray(scale).reshape(-1)[0])
    bias_val = float(np.asarray(bias).reshape(-1)[0]) if np.asarray(bias).size else 0.0

    nc = _get_nc(scale_val, bias_val)
    in_maps = make_in_maps(
        query_input, source_input, Wq_w, Wq_b, Wk_w, Wk_b, Wv_w, Wv_b, scale, bias
    )

    from concourse.bass_utils import run_bass_kernel_spmd

    res = run_bass_kernel_spmd(
        nc, in_maps, core_ids=list(range(N_CORES)), trace=_trace
    )

    out = np.zeros((B, N, D), dtype=np.float32)
    for c in range(N_CORES):
        b = c // 4
        g = c % 4
        r = res.results[c]["out"]  # [2, 256, 64]
        for half in range(2):
            q0 = half * NH + g * QB
            out[b, q0 : q0 + QB, :] = r[half]
    if _trace:
        kernel.last_exec_time_ns = res.exec_time_ns
        kernel.last_results = res
    return out
